# revision 1
# baseline (speedup 1.0000x reference)
"""Causal self-attention (B=4, T=2048, C=1024, NH=16) on 8 TRN2 NeuronCores.

Sharding: core c -> batch b = c//2, head-group g = c%2 (8 heads, Dh=512).
Each core computes q/k/v projections for its head group on its batch,
fused causal attention (attT layout: k on partitions), and a partial
output projection through its row-slice of Wp. Host sums the two
partials per batch.

Device dataflow per core:
  xt [C,T] (host pre-transposed) --f32r--> qt/kt [128,4,T] (Dh on
  partitions, head pair per 128-chunk), v (bf16, per-k-chunk lhsT slots
  with a ones column that makes softmax denominators a free extra psum
  row of the AV matmul). Causal: only lower-triangle k-chunks computed;
  diagonal chunks masked with a host tri mask after exp. Normalization:
  collected s rows -> batch reciprocal -> PE broadcast matmul -> one
  tensor_tensor scale of yt. Out-proj: yt @ wp in f32r.

kernel(**inputs) takes the FULL unsharded inputs and returns the FULL
output. Self-contained: hardcodes all shapes, reads nothing from disk.
"""

import sys

sys.path.insert(0, "/opt/trn_rl_repo")

import numpy as np
import ml_dtypes
from contextlib import ExitStack

import concourse.bass as bass  # noqa: F401  (engine types referenced via nc)
import concourse.mybir as mybir
import concourse.tile as tile
from concourse import bacc
from concourse.bass_utils import run_bass_kernel_spmd

P = 128
B, T, C = 4, 2048, 1024
NH, HS = 16, 64
D = 512          # per-core head dim (8 heads)
H = 8            # local heads
f32 = mybir.dt.float32
f32r = mybir.dt.float32r
bf16 = mybir.dt.bfloat16
AFT = mybir.ActivationFunctionType


def build_nc(t=T, stage="full"):
    """Build the single-core SPMD program (same code, per-core data).

    stage: "A" (projections only), "B" (+attention), "full" — debug aid;
    partial stages dump intermediates into `out` instead of the result.
    """
    assert t % 512 == 0
    nq = t // 512    # q blocks of 512
    nkc = t // 128   # k chunks of 128
    nb = t // 512    # T blocks for projections
    nco = C // P     # C chunks (8)

    nc = bacc.Bacc("TRN2", target_bir_lowering=False, debug=False, num_devices=8)

    xt_d = nc.dram_tensor("xt", [C, t], f32, kind="ExternalInput")
    wq_d = nc.dram_tensor("wq", [C, D], f32, kind="ExternalInput")
    wk_d = nc.dram_tensor("wk", [C, D], f32, kind="ExternalInput")
    wv_d = nc.dram_tensor("wv", [C, D], f32, kind="ExternalInput")
    wp_d = nc.dram_tensor("wp", [D, C], bf16, kind="ExternalInput")
    tri_d = nc.dram_tensor("tri", [P, P], bf16, kind="ExternalInput")
    bcm_d = nc.dram_tensor("bcm", [P, P], f32, kind="ExternalInput")
    out_d = nc.dram_tensor("out", [t, C], f32, kind="ExternalOutput")

    xt_r = xt_d[:].bitcast(f32r).rearrange("(co p) t -> p co t", p=P)
    wq_r = wq_d[:].bitcast(f32r).rearrange("(co p) d -> p co d", p=P)
    wk_r = wk_d[:].bitcast(f32r).rearrange("(co p) d -> p co d", p=P)
    wv_r = wv_d[:].bitcast(f32r).rearrange("(co p) d -> p co d", p=P)
    wp_r = wp_d[:].rearrange("(dc p) c -> p dc c", p=P)
    out_r = out_d[:].rearrange("(tc p) c -> p tc c", p=P)

    with tile.TileContext(nc) as tc, ExitStack() as ctx, nc.allow_low_precision(
        reason="f32r/bf16 attention kernel"
    ):
        # ---- persistent pool: spans projection + attention phases ----
        perm = ctx.enter_context(tc.tile_pool(name="perm", bufs=1))
        psum = ctx.enter_context(tc.tile_pool(name="psum", bufs=2, space="PSUM"))

        qt_sb = perm.tile([P, 4, t], f32r)   # Dh on partitions, head pair/chunk
        kt_sb = perm.tile([P, 4, t], f32r)
        v_sb = perm.tile([P, nkc, H, P], bf16)  # per-chunk AV lhsT slots
        tri_sb = perm.tile([P, P], bf16)
        bcm_sb = perm.tile([P, P], f32)
        nc.sync.dma_start(tri_sb[:], tri_d[:])
        nc.sync.dma_start(bcm_sb[:], bcm_d[:])
        # zero v slots (junk cols would put NaNs in unread psum rows)
        nc.gpsimd.memset(v_sb[:], 0.0)
        # ones columns: even head -> col 64 (sum row 64); odd -> col 0 (row 0)
        v5 = v_sb[:].rearrange("p k (hp par) c -> p k hp par c", par=2)
        nc.gpsimd.memset(v5[:, :, :, 0, 64:65], 1.0)
        nc.gpsimd.memset(v5[:, :, :, 1, 0:1], 1.0)

        # ---- phase A: projections ----
        with tc.tile_pool(name="pha", bufs=1) as pha:
            wq_sb = pha.tile([P, nco, D], f32r, tag="wq")
            wk_sb = pha.tile([P, nco, D], f32r, tag="wk")
            wv_sb = pha.tile([P, nco, D], f32r, tag="wv")
            for co in range(nco):  # per-chunk so matmuls start early
                nc.sync.dma_start(wq_sb[:, co, :], wq_r[:, co, :])
                nc.sync.dma_start(wk_sb[:, co, :], wk_r[:, co, :])
                nc.sync.dma_start(wv_sb[:, co, :], wv_r[:, co, :])
            for ib in range(nb):
                xtb = pha.tile([P, nco, 512], f32r, tag="xtb", bufs=2)
                nc.sync.dma_start(xtb[:], xt_r[:, :, ib * 512 : (ib + 1) * 512])
                for m in range(4):  # qt/kt row chunks of Dh
                    psq = psum.tile([P, 512], f32, tag="pa")
                    for co in range(nco):
                        nc.tensor.matmul(
                            psq[:],
                            wq_sb[:, co, m * P : (m + 1) * P],
                            xtb[:, co, :],
                            start=(co == 0),
                            stop=(co == nco - 1),
                        )
                    nc.vector.tensor_copy(
                        out=qt_sb[:, m, ib * 512 : (ib + 1) * 512], in_=psq[:]
                    )
                    psk = psum.tile([P, 512], f32, tag="pb")
                    for co in range(nco):
                        nc.tensor.matmul(
                            psk[:],
                            wk_sb[:, co, m * P : (m + 1) * P],
                            xtb[:, co, :],
                            start=(co == 0),
                            stop=(co == nco - 1),
                        )
                    nc.vector.tensor_copy(
                        out=kt_sb[:, m, ib * 512 : (ib + 1) * 512], in_=psk[:]
                    )
                for t4 in range(4):  # v chunks of 128 rows within this block
                    kc = ib * 4 + t4
                    psv = psum.tile([P, 512], f32, tag="pc")
                    for co in range(nco):
                        nc.tensor.matmul(
                            psv[:],
                            xtb[:, co, t4 * P : (t4 + 1) * P],
                            wv_sb[:, co, :],
                            start=(co == 0),
                            stop=(co == nco - 1),
                        )
                    # scatter heads into lhsT slots: even -> cols 0:64 of
                    # slot (par 0), odd -> cols 64:128 (par 1)
                    src = psv[:].rearrange("p (hp par c) -> p hp par c", par=2, c=64)
                    nc.vector.tensor_copy(
                        out=v5[:, kc, :, 0, 0:64], in_=src[:, :, 0, :]
                    )
                    nc.vector.tensor_copy(
                        out=v5[:, kc, :, 1, 64:128], in_=src[:, :, 1, :]
                    )

        if stage == "A":
            w_ = min(C, t)
            nc.sync.dma_start(out_r[:, 0, 0:w_], qt_sb[:, 0, 0:w_].bitcast(f32))
            nc.sync.dma_start(out_r[:, 1, 0:w_], kt_sb[:, 0, 0:w_].bitcast(f32))
            with tc.tile_pool(name="dbg", bufs=1) as dbg:
                vf = dbg.tile([P, 8 * P], f32)
                nc.vector.tensor_copy(
                    out=vf[:], in_=v_sb[:, 0, :, :].rearrange("p h c -> p (h c)")
                )
                nc.sync.dma_start(out_r[:, 2, :], vf[:])

        # ---- phase B: attention + normalization; phase C: out-proj ----
        if stage != "A":
          with tc.tile_pool(name="phb", bufs=1) as phb:
            yt_sb = phb.tile([P, 4, t], bf16)
            s_sb = phb.tile([P, 4, t], f32)    # rows 64 (even s) / 0 (odd s)
            nc.gpsimd.memset(s_sb[:], 1.0)     # unused rows must be finite

            def emit_norm(hp):
                # PE broadcast of 1/s via bcm, then one scale TT per block.
                # Emitted one head-pair late so the PE's in-order stream
                # never stalls on the ACT ln/exp chain.
                for jb in range(t // 512):
                    q0 = jb * 512
                    rb = psum.tile([P, 512], f32, tag="pc", bufs=2)
                    nc.tensor.matmul(
                        rb[:], bcm_sb[:], s_sb[:, hp, q0 : q0 + 512],
                        start=True, stop=True,
                    )
                    nc.vector.tensor_mul(
                        out=yt_sb[:, hp, q0 : q0 + 512],
                        in0=yt_sb[:, hp, q0 : q0 + 512],
                        in1=rb[:],
                    )

            for hp in range(4):
                lo, hi = slice(0, 64), slice(64, 128)
                for jq in range(nq):
                    q0 = jq * 512
                    nk = (jq + 1) * 4
                    psyE = psum.tile([P, 512], f32, tag="pe", bufs=1)
                    psyO = psum.tile([P, 512], f32, tag="po", bufs=1)
                    prev = None  # software-pipelined AV emission
                    for kc in range(nk):
                        d = kc - jq * 4
                        off = 128 * d if d >= 0 else 0
                        attA = phb.tile([P, 512], bf16, tag="attA", bufs=6)
                        attB = phb.tile([P, 512], bf16, tag="attB", bufs=6)
                        for att, par, sl in ((attA, 0, lo), (attB, 1, hi)):
                            ps = psum.tile(
                                [P, 512], f32, tag=("pa" if par == 0 else "pb"),
                                bufs=2,
                            )
                            nc.tensor.matmul(
                                ps[:, off:512],
                                kt_sb[sl, hp, kc * P : (kc + 1) * P],
                                qt_sb[sl, hp, q0 + off : q0 + 512],
                                start=True,
                                stop=True,
                            )
                            if off > 0:
                                nc.gpsimd.memset(att[:, 0:off], 0.0)
                            nc.scalar.activation(
                                att[:, off:512], ps[:, off:512], AFT.Exp,
                                scale=0.125,
                            )
                            if d >= 0:
                                nc.vector.tensor_mul(
                                    out=att[:, off : off + P],
                                    in0=att[:, off : off + P],
                                    in1=tri_sb[:],
                                )
                        if prev is not None:
                            pkc, pA, pB = prev
                            nc.tensor.matmul(
                                psyE[:], v_sb[:, pkc, 2 * hp, :], pA[:],
                                start=(pkc == 0), stop=False,
                            )
                            nc.tensor.matmul(
                                psyO[:], v_sb[:, pkc, 2 * hp + 1, :], pB[:],
                                start=(pkc == 0), stop=False,
                            )
                        prev = (kc, attA, attB)
                    pkc, pA, pB = prev
                    nc.tensor.matmul(
                        psyE[:], v_sb[:, pkc, 2 * hp, :], pA[:],
                        start=(pkc == 0), stop=True,
                    )
                    nc.tensor.matmul(
                        psyO[:], v_sb[:, pkc, 2 * hp + 1, :], pB[:],
                        start=(pkc == 0), stop=True,
                    )
                    # yt (unnormalized) + s rows out of the psums
                    nc.vector.tensor_copy(
                        out=yt_sb[lo, hp, q0 : q0 + 512], in_=psyE[0:64, :]
                    )
                    nc.vector.tensor_copy(
                        out=yt_sb[hi, hp, q0 : q0 + 512], in_=psyO[64:128, :]
                    )
                    nc.vector.tensor_copy(
                        out=s_sb[64:65, hp, q0 : q0 + 512], in_=psyE[64:65, :]
                    )
                    nc.vector.tensor_copy(
                        out=s_sb[0:1, hp, q0 : q0 + 512], in_=psyO[0:1, :]
                    )
                # 1/s = exp(-ln s) on ACT (DVE reciprocal is an iterative
                # divide ~6 cyc/elem and stalls the tail; approx_fast
                # miscomputes on HW)
                for row in (slice(0, 1), slice(64, 65)):
                    nc.scalar.activation(
                        s_sb[row, hp, :], s_sb[row, hp, :], AFT.Ln
                    )
                    nc.scalar.activation(
                        s_sb[row, hp, :], s_sb[row, hp, :], AFT.Exp, scale=-1.0
                    )
                if hp > 0:
                    emit_norm(hp - 1)
            emit_norm(3)

            if stage == "B":
                w_ = min(C, t)
                for mm in range(4):
                    dbg_f = phb.tile([P, w_], f32, tag="dbgf", bufs=2)
                    nc.vector.tensor_copy(
                        out=dbg_f[:], in_=yt_sb[:, mm, 0:w_]
                    )
                    nc.sync.dma_start(out_r[:, mm, 0:w_], dbg_f[:])

            # ---- phase C: out = yt.T @ wp ----
            if stage == "full":
              with tc.tile_pool(name="phc", bufs=1) as phc:
                wp_sb = phc.tile([P, 4, C], bf16, tag="wp")
                nc.sync.dma_start(wp_sb[:], wp_r)
                for tcn in range(t // P):
                    ob = phc.tile([P, C], f32, tag="ob", bufs=2)
                    for n2 in range(C // 512):
                        pso = psum.tile([P, 512], f32, tag="pa")
                        for dc in range(4):
                            nc.tensor.matmul(
                                pso[:],
                                yt_sb[:, dc, tcn * P : (tcn + 1) * P],
                                wp_sb[:, dc, n2 * 512 : (n2 + 1) * 512],
                                start=(dc == 0),
                                stop=(dc == 3),
                            )
                        nc.vector.tensor_copy(
                            out=ob[:, n2 * 512 : (n2 + 1) * 512], in_=pso[:]
                        )
                    nc.sync.dma_start(out_r[:, tcn, :], ob[:])

    nc.finalize()
    return nc


_NC = None


def _get_nc():
    global _NC
    if _NC is None:
        _NC = build_nc()
    return _NC


def make_in_maps(x, Wk, Wq, Wv, Wp):
    x = np.asarray(x, dtype=np.float32)
    Wk = np.asarray(Wk, dtype=np.float32)
    Wq = np.asarray(Wq, dtype=np.float32)
    Wv = np.asarray(Wv, dtype=np.float32)
    Wp = np.asarray(Wp, dtype=np.float32)
    tri = np.triu(np.ones((P, P), np.float32)).astype(ml_dtypes.bfloat16)
    bcm = np.zeros((P, P), np.float32)
    bcm[0, 64:128] = 1.0   # odd head r (s at row 0) -> yt rows 64:128
    bcm[64, 0:64] = 1.0    # even head r (s at row 64) -> yt rows 0:64
    in_maps = []
    for c in range(8):
        b, g = c // 2, c % 2
        sl = slice(g * D, (g + 1) * D)
        in_maps.append({
            "xt": np.ascontiguousarray(x[b].T),
            "wq": np.ascontiguousarray(Wq[:, sl]),
            "wk": np.ascontiguousarray(Wk[:, sl]),
            "wv": np.ascontiguousarray(Wv[:, sl]),
            "wp": np.ascontiguousarray(Wp[sl, :]).astype(ml_dtypes.bfloat16),
            "tri": tri,
            "bcm": bcm,
        })
    return in_maps


def _run(x, Wk, Wq, Wv, Wp, trace=False):
    nc = _get_nc()
    in_maps = make_in_maps(x, Wk, Wq, Wv, Wp)
    res = run_bass_kernel_spmd(nc, in_maps, core_ids=list(range(8)), trace=trace)
    parts = [res.results[c]["out"] for c in range(8)]
    out = np.stack(
        [parts[2 * b] + parts[2 * b + 1] for b in range(B)], axis=0
    ).astype(np.float32)
    return out, res


def kernel(x, Wk, Wq, Wv, Wp):
    out, _ = _run(x, Wk, Wq, Wv, Wp, trace=False)
    return out



# revision 8
# speedup vs baseline: 1.1907x; 1.1907x over previous
"""Causal self-attention (B=4, T=2048, C=1024, NH=16) on 8 TRN2 NeuronCores.

Sharding: core c -> batch b = c//2, head-group g = c%2 (8 heads, Dh=512).
Each core computes q/k/v projections for its head group on its batch,
fused causal attention (attT layout: k on partitions), and a partial
output projection through its row-slice of Wp. Host sums the two
partials per batch.

Pipeline structure (per 512-row block ib): projections for block ib ->
attention stage jq=ib over all 4 head pairs -> normalization ->
output projection + store for block ib. The Tile list scheduler fills
PE bubbles of the ACT-bound attention chain with projection/out-proj
matmuls of neighboring stages.

Per-core dataflow: everything bf16 on the matmul paths. QK for the
even/odd head of a pair go to the two banks of one [128,1024] psum
tile (row groups 0-1 / 2-3 run concurrently) so ONE activation
instruction exps both. v is stored per-k-chunk as AV lhsT slots with a
ones column that makes softmax denominators a free extra psum row of
the AV matmul. Causal: only lower-triangle k-chunks computed; QK, exp
and AV all trim to the valid q columns; diagonal chunks masked with a
host tri mask after exp. Normalization: s rows collected on 8
partitions of a tiny tile -> one Ln + one Exp -> K=8 PE broadcast
matmul -> one scale per (hp, block).

kernel(**inputs) takes the FULL unsharded inputs and returns the FULL
output. Self-contained: hardcodes all shapes, reads nothing from disk.
"""

import sys

sys.path.insert(0, "/opt/trn_rl_repo")

import numpy as np
import ml_dtypes
from contextlib import ExitStack

import concourse.bass as bass  # noqa: F401  (engine types referenced via nc)
import concourse.mybir as mybir
import concourse.tile as tile
from concourse import bacc
from concourse.bass_utils import run_bass_kernel_spmd

P = 128
B, T, C = 4, 2048, 1024
NH, HS = 16, 64
D = 512          # per-core head dim (8 heads)
H = 8            # local heads
f32 = mybir.dt.float32
bf16 = mybir.dt.bfloat16
AFT = mybir.ActivationFunctionType


def build_nc(t=T):
    """Build the single-core SPMD program (same code, per-core data)."""
    assert t % 512 == 0
    nb = t // 512    # 512-row blocks (proj blocks == q blocks == stages)
    nkc = t // 128   # k chunks of 128
    nco = C // P     # C chunks (8)

    nc = bacc.Bacc("TRN2", target_bir_lowering=False, debug=False, num_devices=8)

    xt_d = nc.dram_tensor("xt", [C, t], bf16, kind="ExternalInput")
    wq_d = nc.dram_tensor("wq", [C, D], bf16, kind="ExternalInput")
    wk_d = nc.dram_tensor("wk", [C, D], bf16, kind="ExternalInput")
    wv_d = nc.dram_tensor("wv", [C, D], bf16, kind="ExternalInput")
    wp_d = nc.dram_tensor("wp", [D, C], bf16, kind="ExternalInput")
    tri_d = nc.dram_tensor("tri", [P, P], bf16, kind="ExternalInput")
    bcm_d = nc.dram_tensor("bcm", [P, 4, P], bf16, kind="ExternalInput")
    out_d = nc.dram_tensor("out", [t, C], f32, kind="ExternalOutput")

    xt_r = xt_d[:].rearrange("(co p) t -> p co t", p=P)
    wq_r = wq_d[:].rearrange("(co p) d -> p co d", p=P)
    wk_r = wk_d[:].rearrange("(co p) d -> p co d", p=P)
    wv_r = wv_d[:].rearrange("(co p) d -> p co d", p=P)
    wp_r = wp_d[:].rearrange("(dc p) c -> p dc c", p=P)
    out_r = out_d[:].rearrange("(tc p) c -> p tc c", p=P)

    with tile.TileContext(nc) as tc, ExitStack() as ctx, nc.allow_low_precision(
        reason="bf16 attention kernel"
    ):
        perm = ctx.enter_context(tc.tile_pool(name="perm", bufs=1))
        work = ctx.enter_context(tc.tile_pool(name="work", bufs=1))
        psum = ctx.enter_context(tc.tile_pool(name="psum", bufs=1, space="PSUM"))

        qt_sb = perm.tile([P, 4, t], bf16)   # Dh on partitions, head pair/chunk
        kt_sb = perm.tile([P, 4, t], bf16)
        v_sb = perm.tile([P, nkc, H, P], bf16)  # per-chunk AV lhsT slots
        yt_sb = perm.tile([P, 4, t], bf16)
        wq_sb = perm.tile([P, nco, D], bf16)
        wk_sb = perm.tile([P, nco, D], bf16)
        wv_sb = perm.tile([P, nco, D], bf16)
        wp_sb = perm.tile([P, 4, C], bf16)
        tri_sb = perm.tile([P, P], bf16)
        bcm_sb = perm.tile([P, 4, P], bf16)
        # s rows: even head of hp at [64, hp, :], odd at [0, hp, :].
        # Junk partitions 1:64 stay 1.0 so Ln/Exp keep them finite for
        # the K=65 broadcast matmul (scol is never Ln'd in place).
        scol = perm.tile([P, 4, 512], f32)
        scln = perm.tile([P, 4, 512], f32)
        scolr = perm.tile([P, 4, 512], bf16)

        for co in range(nco):  # per-chunk so matmuls can start early
            nc.sync.dma_start(wq_sb[:, co, :], wq_r[:, co, :])
            nc.sync.dma_start(wk_sb[:, co, :], wk_r[:, co, :])
            nc.sync.dma_start(wv_sb[:, co, :], wv_r[:, co, :])
        nc.sync.dma_start(tri_sb[:], tri_d[:])
        nc.sync.dma_start(bcm_sb[:], bcm_d[:])
        nc.sync.dma_start(wp_sb[:], wp_r)
        # zero v slots (junk cols would put NaNs in unread psum rows)
        nc.gpsimd.memset(v_sb[:], 0.0)
        # ones columns: even head -> col 64 (sum row 64); odd -> col 0 (row 0)
        v5 = v_sb[:].rearrange("p k (hp par) c -> p k hp par c", par=2)
        nc.gpsimd.memset(v5[:, :, :, 0, 64:65], 1.0)
        nc.gpsimd.memset(v5[:, :, :, 1, 0:1], 1.0)
        nc.gpsimd.memset(scol[:], 1.0)

        for ib in range(nb):
            b0 = ib * 512
            # ---- projections for block ib ----
            xtb = work.tile([P, nco, 512], bf16, tag="xtb", bufs=2)
            nc.sync.dma_start(xtb[:], xt_r[:, :, b0 : b0 + 512])
            for m in range(4):  # qt/kt row chunks of Dh
                psq = psum.tile([P, 512], f32, tag="gen", bufs=2)
                for co in range(nco):
                    nc.tensor.matmul(
                        psq[:],
                        wq_sb[:, co, m * P : (m + 1) * P],
                        xtb[:, co, :],
                        start=(co == 0),
                        stop=(co == nco - 1),
                    )
                nc.vector.tensor_copy(out=qt_sb[:, m, b0 : b0 + 512], in_=psq[:])
                psk = psum.tile([P, 512], f32, tag="gen", bufs=2)
                for co in range(nco):
                    nc.tensor.matmul(
                        psk[:],
                        wk_sb[:, co, m * P : (m + 1) * P],
                        xtb[:, co, :],
                        start=(co == 0),
                        stop=(co == nco - 1),
                    )
                nc.vector.tensor_copy(out=kt_sb[:, m, b0 : b0 + 512], in_=psk[:])
            for t4 in range(4):  # v chunks of 128 rows within this block
                kc = ib * 4 + t4
                psv = psum.tile([P, 512], f32, tag="gen", bufs=2)
                for co in range(nco):
                    nc.tensor.matmul(
                        psv[:],
                        xtb[:, co, t4 * P : (t4 + 1) * P],
                        wv_sb[:, co, :],
                        start=(co == 0),
                        stop=(co == nco - 1),
                    )
                # scatter heads into lhsT slots: even -> cols 0:64 of
                # slot (par 0), odd -> cols 64:128 (par 1)
                src = psv[:].rearrange("p (hp par c) -> p hp par c", par=2, c=64)
                nc.vector.tensor_copy(out=v5[:, kc, :, 0, 0:64], in_=src[:, :, 0, :])
                nc.vector.tensor_copy(out=v5[:, kc, :, 1, 64:128], in_=src[:, :, 1, :])

            # ---- attention stage jq = ib ----
            nk = (ib + 1) * 4
            lo, hi = slice(0, 64), slice(64, 128)
            for hp in range(4):
                pav = psum.tile([P, 1024], f32, tag="av", bufs=1)
                for kc in range(nk):
                    d = kc - ib * 4
                    off = 128 * d if d >= 0 else 0
                    pqk = psum.tile([P, 1024], f32, tag="qk", bufs=2)
                    nc.tensor.matmul(
                        pqk[:, off:512],
                        kt_sb[lo, hp, kc * P : (kc + 1) * P],
                        qt_sb[lo, hp, b0 + off : b0 + 512],
                        start=True,
                        stop=True,
                    )
                    nc.tensor.matmul(
                        pqk[:, 512 + off : 1024],
                        kt_sb[hi, hp, kc * P : (kc + 1) * P],
                        qt_sb[hi, hp, b0 + off : b0 + 512],
                        start=True,
                        stop=True,
                    )
                    att = work.tile([P, 1024], bf16, tag="att", bufs=4)
                    # one exp over both heads; [512, 512+off) is unwritten
                    # psum junk but lands in att cols the trimmed AV never
                    # reads
                    nc.scalar.activation(
                        att[:, off:1024], pqk[:, off:1024], AFT.Exp, scale=0.125
                    )
                    if d >= 0:
                        nc.vector.tensor_mul(
                            out=att[:, off : off + P],
                            in0=att[:, off : off + P],
                            in1=tri_sb[:],
                        )
                        nc.vector.tensor_mul(
                            out=att[:, 512 + off : 512 + off + P],
                            in0=att[:, 512 + off : 512 + off + P],
                            in1=tri_sb[:],
                        )
                    nc.tensor.matmul(
                        pav[:, off:512],
                        v_sb[:, kc, 2 * hp, :],
                        att[:, off:512],
                        start=(kc == 0),
                        stop=(kc == nk - 1),
                    )
                    nc.tensor.matmul(
                        pav[:, 512 + off : 1024],
                        v_sb[:, kc, 2 * hp + 1, :],
                        att[:, 512 + off : 1024],
                        start=(kc == 0),
                        stop=(kc == nk - 1),
                    )
                # yt (unnormalized) + s rows out of the psum halves
                nc.vector.tensor_copy(
                    out=yt_sb[lo, hp, b0 : b0 + 512], in_=pav[0:64, 0:512]
                )
                nc.vector.tensor_copy(
                    out=yt_sb[hi, hp, b0 : b0 + 512], in_=pav[64:128, 512:1024]
                )
                nc.vector.tensor_copy(
                    out=scol[64:65, hp, :], in_=pav[64:65, 0:512]
                )
                nc.vector.tensor_copy(
                    out=scol[0:1, hp, :], in_=pav[0:1, 512:1024]
                )
            # 1/s = exp(-ln s); one pass over partitions 0:65 covers both
            # s rows, cost is free-dim driven (DVE reciprocal is an
            # iterative divide and approx_fast miscomputes on HW)
            nc.scalar.activation(scln[0:65, :, :], scol[0:65, :, :], AFT.Ln)
            nc.scalar.activation(
                scolr[0:65, :, :], scln[0:65, :, :], AFT.Exp, scale=-1.0
            )
            for hp in range(4):
                rb = psum.tile([P, 512], f32, tag="gen", bufs=2)
                nc.tensor.matmul(
                    rb[:], bcm_sb[0:65, hp, :], scolr[0:65, hp, :],
                    start=True, stop=True,
                )
                nc.vector.tensor_mul(
                    out=yt_sb[:, hp, b0 : b0 + 512],
                    in0=yt_sb[:, hp, b0 : b0 + 512],
                    in1=rb[:],
                )

            # ---- out-proj + store for block ib ----
            for t4 in range(4):
                tcn = ib * 4 + t4
                ob = work.tile([P, C], f32, tag="ob", bufs=2)
                for n2 in range(C // 512):
                    pso = psum.tile([P, 512], f32, tag="gen", bufs=2)
                    for dc in range(4):
                        nc.tensor.matmul(
                            pso[:],
                            yt_sb[:, dc, tcn * P : (tcn + 1) * P],
                            wp_sb[:, dc, n2 * 512 : (n2 + 1) * 512],
                            start=(dc == 0),
                            stop=(dc == 3),
                        )
                    nc.vector.tensor_copy(
                        out=ob[:, n2 * 512 : (n2 + 1) * 512], in_=pso[:]
                    )
                nc.sync.dma_start(out_r[:, tcn, :], ob[:])

    nc.finalize()
    return nc


_NC = None


def _get_nc():
    global _NC
    if _NC is None:
        _NC = build_nc()
    return _NC


def make_in_maps(x, Wk, Wq, Wv, Wp):
    x = np.asarray(x, dtype=np.float32)
    Wk = np.asarray(Wk, dtype=np.float32)
    Wq = np.asarray(Wq, dtype=np.float32)
    Wv = np.asarray(Wv, dtype=np.float32)
    Wp = np.asarray(Wp, dtype=np.float32)
    tri = np.triu(np.ones((P, P), np.float32)).astype(ml_dtypes.bfloat16)
    # bcm broadcasts 1/s rows (even at 64, odd at 0) to yt rows
    bcm = np.zeros((P, 4, P), np.float32)
    for hp in range(4):
        bcm[64, hp, 0:64] = 1.0
        bcm[0, hp, 64:128] = 1.0
    bcm = bcm.astype(ml_dtypes.bfloat16)
    in_maps = []
    for c in range(8):
        b, g = c // 2, c % 2
        sl = slice(g * D, (g + 1) * D)
        in_maps.append({
            "xt": np.ascontiguousarray(x[b].T).astype(ml_dtypes.bfloat16),
            "wq": np.ascontiguousarray(Wq[:, sl]).astype(ml_dtypes.bfloat16),
            "wk": np.ascontiguousarray(Wk[:, sl]).astype(ml_dtypes.bfloat16),
            "wv": np.ascontiguousarray(Wv[:, sl]).astype(ml_dtypes.bfloat16),
            "wp": np.ascontiguousarray(Wp[sl, :]).astype(ml_dtypes.bfloat16),
            "tri": tri,
            "bcm": bcm,
        })
    return in_maps


def _run(x, Wk, Wq, Wv, Wp, trace=False):
    nc = _get_nc()
    in_maps = make_in_maps(x, Wk, Wq, Wv, Wp)
    res = run_bass_kernel_spmd(nc, in_maps, core_ids=list(range(8)), trace=trace)
    parts = [res.results[c]["out"] for c in range(8)]
    out = np.stack(
        [parts[2 * b] + parts[2 * b + 1] for b in range(B)], axis=0
    ).astype(np.float32)
    return out, res


def kernel(x, Wk, Wq, Wv, Wp):
    out, _ = _run(x, Wk, Wq, Wv, Wp, trace=False)
    return out


# revision 16
# speedup vs baseline: 1.3650x; 1.1463x over previous
"""Causal self-attention (B=4, T=2048, C=1024, NH=16) on 8 TRN2 NeuronCores.

Sharding: core c -> batch b = c//2, head-group g = c%2 (8 heads, Dh=512).
Each core computes q/k/v projections for its head group on its batch,
fused causal attention (attT layout: k on partitions), and a partial
output projection through its row-slice of Wp. Host sums the two
partials per batch.

Pipeline structure (per 512-row block ib): projections for block ib ->
attention stage jq=ib over all 4 head pairs -> normalization ->
output projection + store for block ib. The Tile list scheduler fills
PE bubbles of the ACT-bound attention chain with projection/out-proj
matmuls of neighboring stages.

Per-core dataflow: everything bf16 on the matmul paths. QK for the
even/odd head of a pair go to the two banks of one [128,1024] psum
tile (row groups 0-1 / 2-3 run concurrently) so ONE activation
instruction exps both. v is stored per-k-chunk as AV lhsT slots with a
ones column that makes softmax denominators a free extra psum row of
the AV matmul. Causal: only lower-triangle k-chunks computed; QK, exp
and AV all trim to the valid q columns; diagonal chunks masked with a
host tri mask after exp. Normalization: s rows collected on 8
partitions of a tiny tile -> one Ln + one Exp -> K=8 PE broadcast
matmul -> one scale per (hp, block).

kernel(**inputs) takes the FULL unsharded inputs and returns the FULL
output. Self-contained: hardcodes all shapes, reads nothing from disk.
"""

import sys

sys.path.insert(0, "/opt/trn_rl_repo")

import numpy as np
import ml_dtypes
from contextlib import ExitStack

import concourse.bass as bass  # noqa: F401  (engine types referenced via nc)
import concourse.mybir as mybir
import concourse.tile as tile
from concourse import bacc
from concourse.bass_utils import run_bass_kernel_spmd

P = 128
B, T, C = 4, 2048, 1024
NH, HS = 16, 64
D = 512          # per-core head dim (8 heads)
H = 8            # local heads
f32 = mybir.dt.float32
bf16 = mybir.dt.bfloat16
AFT = mybir.ActivationFunctionType


def build_nc(t=T):
    """Build the single-core SPMD program (same code, per-core data)."""
    assert t % 512 == 0
    nb = t // 512    # 512-row blocks (proj blocks == q blocks == stages)
    nkc = t // 128   # k chunks of 128
    nco = C // P     # C chunks (8)

    nc = bacc.Bacc("TRN2", target_bir_lowering=False, debug=False, num_devices=8)

    xt_d = nc.dram_tensor("xt", [C, t], bf16, kind="ExternalInput")
    wq_d = nc.dram_tensor("wq", [C, D], bf16, kind="ExternalInput")
    wk_d = nc.dram_tensor("wk", [C, D], bf16, kind="ExternalInput")
    wv_d = nc.dram_tensor("wv", [C, D], bf16, kind="ExternalInput")
    wp_d = nc.dram_tensor("wp", [D, C], bf16, kind="ExternalInput")
    tri_d = nc.dram_tensor("tri", [P, P], bf16, kind="ExternalInput")
    bcm_d = nc.dram_tensor("bcm", [P, 4, P], bf16, kind="ExternalInput")
    out_d = nc.dram_tensor("out", [t, C], f32, kind="ExternalOutput")

    xt_r = xt_d[:].rearrange("(co p) t -> p co t", p=P)
    wq_r = wq_d[:].rearrange("(co p) d -> p co d", p=P)
    wk_r = wk_d[:].rearrange("(co p) d -> p co d", p=P)
    wv_r = wv_d[:].rearrange("(co p) d -> p co d", p=P)
    wp_r = wp_d[:].rearrange("(dc p) c -> p dc c", p=P)
    out_r = out_d[:].rearrange("(tc p) c -> p tc c", p=P)

    with tile.TileContext(nc) as tc, ExitStack() as ctx, nc.allow_low_precision(
        reason="bf16 attention kernel"
    ):
        perm = ctx.enter_context(tc.tile_pool(name="perm", bufs=1))
        work = ctx.enter_context(tc.tile_pool(name="work", bufs=1))
        psum = ctx.enter_context(tc.tile_pool(name="psum", bufs=1, space="PSUM"))

        # per-block tiles so stage ib's reads and proj ib+1's writes are
        # on different tiles (no false WAR edges to serialize stages)
        qt_t = [perm.tile([P, 4, 512], bf16, name=f"qt{i}") for i in range(nb)]
        kt_t = [perm.tile([P, 4, 512], bf16, name=f"kt{i}") for i in range(nb)]
        v_t = [perm.tile([P, 4, H, P], bf16, name=f"v{i}") for i in range(nb)]
        yt_t = [perm.tile([P, 4, 512], bf16, name=f"yt{i}") for i in range(nb)]
        wq_sb = perm.tile([P, nco, D], bf16)
        wk_sb = perm.tile([P, nco, D], bf16)
        wv_sb = perm.tile([P, nco, D], bf16)
        wp_sb = perm.tile([P, 4, C], bf16)
        tri_sb = perm.tile([P, P], bf16)
        bcm_sb = perm.tile([P, 4, P], bf16)
        # s rows: even head of hp at [64, hp, :], odd at [0, hp, :].
        # Junk partitions 1:64 stay 1.0 so Ln/Exp keep them finite for
        # the K=65 broadcast matmul (scol is never Ln'd in place).
        scol = perm.tile([P, 4, 512], f32)
        scln = perm.tile([P, 4, 512], f32)
        scolr = perm.tile([P, 4, 512], bf16)

        for co in range(nco):  # per-chunk so matmuls can start early
            nc.sync.dma_start(wq_sb[:, co, :], wq_r[:, co, :])
            nc.sync.dma_start(wk_sb[:, co, :], wk_r[:, co, :])
            nc.sync.dma_start(wv_sb[:, co, :], wv_r[:, co, :])
        nc.sync.dma_start(tri_sb[:], tri_d[:])
        nc.sync.dma_start(bcm_sb[:], bcm_d[:])
        nc.sync.dma_start(wp_sb[:], wp_r)
        # zero v slots (junk cols would put NaNs in unread psum rows);
        # ones columns: even head -> col 64 (sum row 64); odd -> col 0 (row 0)
        v5s = []
        for vb in v_t:
            nc.gpsimd.memset(vb[:], 0.0)
            v5 = vb[:].rearrange("p k (hp par) c -> p k hp par c", par=2)
            nc.gpsimd.memset(v5[:, :, :, 0, 64:65], 1.0)
            nc.gpsimd.memset(v5[:, :, :, 1, 0:1], 1.0)
            v5s.append(v5)
        nc.gpsimd.memset(scol[:], 1.0)

        for ib in range(nb):
            b0 = ib * 512
            # ---- projections for block ib ----
            xtb = work.tile([P, nco, 512], bf16, tag="xtb", bufs=2)
            nc.sync.dma_start(xtb[:], xt_r[:, :, b0 : b0 + 512])
            for m in range(4):  # qt/kt row chunks of Dh
                psq = psum.tile([P, 512], f32, tag="gen", bufs=2)
                for co in range(nco):
                    nc.tensor.matmul(
                        psq[:],
                        wq_sb[:, co, m * P : (m + 1) * P],
                        xtb[:, co, :],
                        start=(co == 0),
                        stop=(co == nco - 1),
                    )
                nc.vector.tensor_copy(out=qt_t[ib][:, m, :], in_=psq[:])
                psk = psum.tile([P, 512], f32, tag="gen", bufs=2)
                for co in range(nco):
                    nc.tensor.matmul(
                        psk[:],
                        wk_sb[:, co, m * P : (m + 1) * P],
                        xtb[:, co, :],
                        start=(co == 0),
                        stop=(co == nco - 1),
                    )
                nc.vector.tensor_copy(out=kt_t[ib][:, m, :], in_=psk[:])
            for t4 in range(4):  # v chunks of 128 rows within this block
                psv = psum.tile([P, 512], f32, tag="gen", bufs=2)
                for co in range(nco):
                    nc.tensor.matmul(
                        psv[:],
                        xtb[:, co, t4 * P : (t4 + 1) * P],
                        wv_sb[:, co, :],
                        start=(co == 0),
                        stop=(co == nco - 1),
                    )
                # scatter heads into lhsT slots: even -> cols 0:64 of
                # slot (par 0), odd -> cols 64:128 (par 1)
                src = psv[:].rearrange("p (hp par c) -> p hp par c", par=2, c=64)
                nc.vector.tensor_copy(
                    out=v5s[ib][:, t4, :, 0, 0:64], in_=src[:, :, 0, :]
                )
                nc.vector.tensor_copy(
                    out=v5s[ib][:, t4, :, 1, 64:128], in_=src[:, :, 1, :]
                )

            # ---- attention stage jq = ib ----
            nk = (ib + 1) * 4
            lo, hi = slice(0, 64), slice(64, 128)
            for hp in range(4):
                pav = psum.tile([P, 1024], f32, tag="av", bufs=1)
                for kc in range(nk):
                    d = kc - ib * 4
                    off = 128 * d if d >= 0 else 0
                    ktb, kcl = kt_t[kc // 4], kc % 4
                    pqk = psum.tile([P, 1024], f32, tag="qk", bufs=2)
                    nc.tensor.matmul(
                        pqk[:, off:512],
                        ktb[lo, hp, kcl * P : (kcl + 1) * P],
                        qt_t[ib][lo, hp, off:512],
                        start=True,
                        stop=True,
                    )
                    nc.tensor.matmul(
                        pqk[:, 512 + off : 1024],
                        ktb[hi, hp, kcl * P : (kcl + 1) * P],
                        qt_t[ib][hi, hp, off:512],
                        start=True,
                        stop=True,
                    )
                    att = work.tile([P, 1024], bf16, tag="att", bufs=4)
                    # one exp over both heads; [512, 512+off) is unwritten
                    # psum junk but lands in att cols the trimmed AV never
                    # reads
                    nc.scalar.activation(
                        att[:, off:1024], pqk[:, off:1024], AFT.Exp, scale=0.125
                    )
                    if d >= 0:
                        nc.vector.tensor_mul(
                            out=att[:, off : off + P],
                            in0=att[:, off : off + P],
                            in1=tri_sb[:],
                        )
                        nc.vector.tensor_mul(
                            out=att[:, 512 + off : 512 + off + P],
                            in0=att[:, 512 + off : 512 + off + P],
                            in1=tri_sb[:],
                        )
                    nc.tensor.matmul(
                        pav[:, off:512],
                        v_t[kc // 4][:, kc % 4, 2 * hp, :],
                        att[:, off:512],
                        start=(kc == 0),
                        stop=(kc == nk - 1),
                    )
                    nc.tensor.matmul(
                        pav[:, 512 + off : 1024],
                        v_t[kc // 4][:, kc % 4, 2 * hp + 1, :],
                        att[:, 512 + off : 1024],
                        start=(kc == 0),
                        stop=(kc == nk - 1),
                    )
                # yt (unnormalized) + s rows out of the psum halves
                nc.vector.tensor_copy(
                    out=yt_t[ib][lo, hp, :], in_=pav[0:64, 0:512]
                )
                nc.vector.tensor_copy(
                    out=yt_t[ib][hi, hp, :], in_=pav[64:128, 512:1024]
                )
                nc.vector.tensor_copy(
                    out=scol[64:65, hp, :], in_=pav[64:65, 0:512]
                )
                nc.vector.tensor_copy(
                    out=scol[0:1, hp, :], in_=pav[0:1, 512:1024]
                )
            # 1/s = exp(-ln s); one pass over partitions 0:65 covers both
            # s rows, cost is free-dim driven (DVE reciprocal is an
            # iterative divide and approx_fast miscomputes on HW)
            nc.scalar.activation(scln[0:65, :, :], scol[0:65, :, :], AFT.Ln)
            nc.scalar.activation(
                scolr[0:65, :, :], scln[0:65, :, :], AFT.Exp, scale=-1.0
            )
            for hp in range(4):
                rb = psum.tile([P, 1024], f32, tag="qk", bufs=2)
                nc.tensor.matmul(
                    rb[:, 0:512], bcm_sb[0:65, hp, :], scolr[0:65, hp, :],
                    start=True, stop=True,
                )
                nc.vector.tensor_mul(
                    out=yt_t[ib][:, hp, :],
                    in0=yt_t[ib][:, hp, :],
                    in1=rb[:, 0:512],
                )

            # ---- out-proj + store for block ib ----
            for t4 in range(4):
                tcn = ib * 4 + t4
                ob = work.tile([P, C], f32, tag="ob", bufs=2)
                pso = psum.tile([P, 1024], f32, tag="qk", bufs=2)
                for n2 in range(C // 512):
                    for dc in range(4):
                        nc.tensor.matmul(
                            pso[:, n2 * 512 : (n2 + 1) * 512],
                            yt_t[ib][:, dc, t4 * P : (t4 + 1) * P],
                            wp_sb[:, dc, n2 * 512 : (n2 + 1) * 512],
                            start=(dc == 0),
                            stop=(dc == 3),
                        )
                nc.vector.tensor_copy(out=ob[:], in_=pso[:])
                nc.sync.dma_start(out_r[:, tcn, :], ob[:])

    nc.finalize()
    return nc


_NC = None


def _get_nc():
    global _NC
    if _NC is None:
        _NC = build_nc()
    return _NC


def make_in_maps(x, Wk, Wq, Wv, Wp):
    x = np.asarray(x, dtype=np.float32)
    Wk = np.asarray(Wk, dtype=np.float32)
    Wq = np.asarray(Wq, dtype=np.float32)
    Wv = np.asarray(Wv, dtype=np.float32)
    Wp = np.asarray(Wp, dtype=np.float32)
    tri = np.triu(np.ones((P, P), np.float32)).astype(ml_dtypes.bfloat16)
    # bcm broadcasts 1/s rows (even at 64, odd at 0) to yt rows
    bcm = np.zeros((P, 4, P), np.float32)
    for hp in range(4):
        bcm[64, hp, 0:64] = 1.0
        bcm[0, hp, 64:128] = 1.0
    bcm = bcm.astype(ml_dtypes.bfloat16)
    in_maps = []
    for c in range(8):
        b, g = c // 2, c % 2
        sl = slice(g * D, (g + 1) * D)
        in_maps.append({
            "xt": np.ascontiguousarray(x[b].T).astype(ml_dtypes.bfloat16),
            "wq": np.ascontiguousarray(Wq[:, sl]).astype(ml_dtypes.bfloat16),
            "wk": np.ascontiguousarray(Wk[:, sl]).astype(ml_dtypes.bfloat16),
            "wv": np.ascontiguousarray(Wv[:, sl]).astype(ml_dtypes.bfloat16),
            "wp": np.ascontiguousarray(Wp[sl, :]).astype(ml_dtypes.bfloat16),
            "tri": tri,
            "bcm": bcm,
        })
    return in_maps


def _run(x, Wk, Wq, Wv, Wp, trace=False):
    nc = _get_nc()
    in_maps = make_in_maps(x, Wk, Wq, Wv, Wp)
    res = run_bass_kernel_spmd(nc, in_maps, core_ids=list(range(8)), trace=trace)
    parts = [res.results[c]["out"] for c in range(8)]
    out = np.stack(
        [parts[2 * b] + parts[2 * b + 1] for b in range(B)], axis=0
    ).astype(np.float32)
    return out, res


def kernel(x, Wk, Wq, Wv, Wp):
    out, _ = _run(x, Wk, Wq, Wv, Wp, trace=False)
    return out


# revision 21
# speedup vs baseline: 1.4218x; 1.0417x over previous
"""Causal self-attention (B=4, T=2048, C=1024, NH=16) on 8 TRN2 NeuronCores.

Sharding: core c -> batch b = c//2, head-group g = c%2 (8 heads, Dh=512).
Each core computes q/k/v projections for its head group on its batch,
fused causal attention (attT layout: k on partitions), and a partial
output projection through its row-slice of Wp. Host sums the two
partials per batch.

Pipeline structure (per 512-row block ib): projections for block ib ->
attention stage jq=ib over all 4 head pairs -> normalization ->
output projection + store for block ib. The Tile list scheduler fills
PE bubbles of the ACT-bound attention chain with projection/out-proj
matmuls of neighboring stages.

Per-core dataflow: everything bf16 on the matmul paths. QK for the
even/odd head of a pair go to the two banks of one [128,1024] psum
tile (row groups 0-1 / 2-3 run concurrently) so ONE activation
instruction exps both. v is stored per-k-chunk as AV lhsT slots with a
ones column that makes softmax denominators a free extra psum row of
the AV matmul. Causal: only lower-triangle k-chunks computed; QK, exp
and AV all trim to the valid q columns; diagonal chunks masked with a
host tri mask after exp. Normalization: s rows collected on 8
partitions of a tiny tile -> one Ln + one Exp -> K=8 PE broadcast
matmul -> one scale per (hp, block).

kernel(**inputs) takes the FULL unsharded inputs and returns the FULL
output. Self-contained: hardcodes all shapes, reads nothing from disk.
"""

import sys

sys.path.insert(0, "/opt/trn_rl_repo")

import numpy as np
import ml_dtypes
from contextlib import ExitStack

import concourse.bass as bass  # noqa: F401  (engine types referenced via nc)
import concourse.mybir as mybir
import concourse.tile as tile
from concourse import bacc
from concourse.bass_utils import run_bass_kernel_spmd

P = 128
B, T, C = 4, 2048, 1024
NH, HS = 16, 64
D = 512          # per-core head dim (8 heads)
H = 8            # local heads
f32 = mybir.dt.float32
bf16 = mybir.dt.bfloat16
AFT = mybir.ActivationFunctionType


def build_nc(t=T):
    """Build the single-core SPMD program (same code, per-core data)."""
    assert t % 512 == 0
    nb = t // 512    # 512-row blocks (proj blocks == q blocks == stages)
    nkc = t // 128   # k chunks of 128
    nco = C // P     # C chunks (8)

    nc = bacc.Bacc("TRN2", target_bir_lowering=False, debug=False, num_devices=8)

    xt_d = nc.dram_tensor("xt", [C, t], bf16, kind="ExternalInput")
    wq_d = nc.dram_tensor("wq", [C, D], bf16, kind="ExternalInput")
    wk_d = nc.dram_tensor("wk", [C, D], bf16, kind="ExternalInput")
    wv_d = nc.dram_tensor("wv", [C, D], bf16, kind="ExternalInput")
    wp_d = nc.dram_tensor("wp", [D, C], bf16, kind="ExternalInput")
    tri_d = nc.dram_tensor("tri", [P, P], bf16, kind="ExternalInput")
    bcm_d = nc.dram_tensor("bcm", [P, 4, P], bf16, kind="ExternalInput")
    out_d = nc.dram_tensor("out", [t, C], f32, kind="ExternalOutput")

    xt_r = xt_d[:].rearrange("(co p) t -> p co t", p=P)
    wq_r = wq_d[:].rearrange("(co p) d -> p co d", p=P)
    wk_r = wk_d[:].rearrange("(co p) d -> p co d", p=P)
    wv_r = wv_d[:].rearrange("(co p) d -> p co d", p=P)
    wp_r = wp_d[:].rearrange("(dc p) c -> p dc c", p=P)
    out_r = out_d[:].rearrange("(tc p) c -> p tc c", p=P)

    with tile.TileContext(nc) as tc, ExitStack() as ctx, nc.allow_low_precision(
        reason="bf16 attention kernel"
    ):
        perm = ctx.enter_context(tc.tile_pool(name="perm", bufs=1))
        work = ctx.enter_context(tc.tile_pool(name="work", bufs=1))
        psum = ctx.enter_context(tc.tile_pool(name="psum", bufs=1, space="PSUM"))

        # per-block tiles so stage ib's reads and proj ib+1's writes are
        # on different tiles (no false WAR edges to serialize stages)
        qt_t = [perm.tile([P, 4, 512], bf16, name=f"qt{i}") for i in range(nb)]
        kt_t = [perm.tile([P, 4, 512], bf16, name=f"kt{i}") for i in range(nb)]
        v_t = [perm.tile([P, 4, H, P], bf16, name=f"v{i}") for i in range(nb)]
        yt_t = [perm.tile([P, 4, 512], bf16, name=f"yt{i}") for i in range(nb)]
        wq_sb = perm.tile([P, nco, D], bf16)
        wk_sb = perm.tile([P, nco, D], bf16)
        wv_sb = perm.tile([P, nco, D], bf16)
        wp_sb = perm.tile([P, 4, C], bf16)
        tri_sb = perm.tile([P, P], bf16)
        bcm_sb = perm.tile([P, 4, P], bf16)
        # s rows: even head of hp at [64, hp, :], odd at [0, hp, :].
        # Junk partitions 1:64 stay 1.0 so the reciprocal keeps them
        # finite for the K=65 broadcast matmul.
        scol = perm.tile([P, 4, 512], f32)
        scolf = perm.tile([P, 4, 512], f32)
        scolr = perm.tile([P, 4, 512], bf16)

        # DMA order: first q-proj chain needs only wq + x block 0
        nc.sync.dma_start(wq_sb[:], wq_r)
        xtb0 = work.tile([P, nco, 512], bf16, tag="xtb", bufs=2)
        nc.sync.dma_start(xtb0[:], xt_r[:, :, 0:512])
        nc.sync.dma_start(wk_sb[:], wk_r)
        nc.sync.dma_start(wv_sb[:], wv_r)
        nc.sync.dma_start(tri_sb[:], tri_d[:])
        nc.sync.dma_start(bcm_sb[:], bcm_d[:])
        nc.sync.dma_start(wp_sb[:], wp_r)
        # zero v slots (junk cols would put NaNs in unread psum rows);
        # ones columns: even head -> col 64 (sum row 64); odd -> col 0 (row 0)
        v5s = []
        for vb in v_t:
            nc.gpsimd.memset(vb[:], 0.0)
            v5 = vb[:].rearrange("p k (hp par) c -> p k hp par c", par=2)
            nc.gpsimd.memset(v5[:, :, :, 0, 64:65], 1.0)
            nc.gpsimd.memset(v5[:, :, :, 1, 0:1], 1.0)
            v5s.append(v5)
        nc.gpsimd.memset(scol[:], 1.0)

        outproj_deferred = []
        for ib in range(nb):
            b0 = ib * 512
            # ---- projections for block ib ----
            if ib == 0:
                xtb = xtb0
            else:
                xtb = work.tile([P, nco, 512], bf16, tag="xtb", bufs=2)
                nc.sync.dma_start(xtb[:], xt_r[:, :, b0 : b0 + 512])
            for m in range(4):  # qt/kt row chunks of Dh
                psq = psum.tile([P, 512], f32, tag="gen", bufs=2)
                for co in range(nco):
                    nc.tensor.matmul(
                        psq[:],
                        wq_sb[:, co, m * P : (m + 1) * P],
                        xtb[:, co, :],
                        start=(co == 0),
                        stop=(co == nco - 1),
                    )
                nc.vector.tensor_copy(out=qt_t[ib][:, m, :], in_=psq[:])
                psk = psum.tile([P, 512], f32, tag="gen", bufs=2)
                for co in range(nco):
                    nc.tensor.matmul(
                        psk[:],
                        wk_sb[:, co, m * P : (m + 1) * P],
                        xtb[:, co, :],
                        start=(co == 0),
                        stop=(co == nco - 1),
                    )
                nc.vector.tensor_copy(out=kt_t[ib][:, m, :], in_=psk[:])
            for t4 in range(4):  # v chunks of 128 rows within this block
                psv = psum.tile([P, 512], f32, tag="gen", bufs=2)
                for co in range(nco):
                    nc.tensor.matmul(
                        psv[:],
                        xtb[:, co, t4 * P : (t4 + 1) * P],
                        wv_sb[:, co, :],
                        start=(co == 0),
                        stop=(co == nco - 1),
                    )
                # scatter heads into lhsT slots: even -> cols 0:64 of
                # slot (par 0), odd -> cols 64:128 (par 1)
                src = psv[:].rearrange("p (hp par c) -> p hp par c", par=2, c=64)
                nc.vector.tensor_copy(
                    out=v5s[ib][:, t4, :, 0, 0:64], in_=src[:, :, 0, :]
                )
                nc.vector.tensor_copy(
                    out=v5s[ib][:, t4, :, 1, 64:128], in_=src[:, :, 1, :]
                )

            # ---- attention stage jq = ib ----
            nk = (ib + 1) * 4
            lo, hi = slice(0, 64), slice(64, 128)
            for hp in range(4):
                pav = psum.tile([P, 1024], f32, tag="av", bufs=1)
                for kc in range(nk):
                    d = kc - ib * 4
                    off = 128 * d if d >= 0 else 0
                    ktb, kcl = kt_t[kc // 4], kc % 4
                    pqk = psum.tile([P, 1024], f32, tag="qk", bufs=2)
                    nc.tensor.matmul(
                        pqk[:, off:512],
                        ktb[lo, hp, kcl * P : (kcl + 1) * P],
                        qt_t[ib][lo, hp, off:512],
                        start=True,
                        stop=True,
                    )
                    nc.tensor.matmul(
                        pqk[:, 512 + off : 1024],
                        ktb[hi, hp, kcl * P : (kcl + 1) * P],
                        qt_t[ib][hi, hp, off:512],
                        start=True,
                        stop=True,
                    )
                    att = work.tile([P, 1024], bf16, tag="att", bufs=4)
                    # one exp over both heads; [512, 512+off) is unwritten
                    # psum junk but lands in att cols the trimmed AV never
                    # reads
                    nc.scalar.activation(
                        att[:, off:1024], pqk[:, off:1024], AFT.Exp, scale=0.125
                    )
                    if d >= 0:
                        # gpsimd keeps the masked muls off the busier DVE
                        nc.gpsimd.tensor_mul(
                            out=att[:, off : off + P],
                            in0=att[:, off : off + P],
                            in1=tri_sb[:],
                        )
                        nc.gpsimd.tensor_mul(
                            out=att[:, 512 + off : 512 + off + P],
                            in0=att[:, 512 + off : 512 + off + P],
                            in1=tri_sb[:],
                        )
                    nc.tensor.matmul(
                        pav[:, off:512],
                        v_t[kc // 4][:, kc % 4, 2 * hp, :],
                        att[:, off:512],
                        start=(kc == 0),
                        stop=(kc == nk - 1),
                    )
                    nc.tensor.matmul(
                        pav[:, 512 + off : 1024],
                        v_t[kc // 4][:, kc % 4, 2 * hp + 1, :],
                        att[:, 512 + off : 1024],
                        start=(kc == 0),
                        stop=(kc == nk - 1),
                    )
                # yt (unnormalized) + s rows out of the psum halves
                nc.vector.tensor_copy(
                    out=yt_t[ib][lo, hp, :], in_=pav[0:64, 0:512]
                )
                nc.vector.tensor_copy(
                    out=yt_t[ib][hi, hp, :], in_=pav[64:128, 512:1024]
                )
                nc.vector.tensor_copy(
                    out=scol[64:65, hp, :], in_=pav[64:65, 0:512]
                )
                nc.vector.tensor_copy(
                    out=scol[0:1, hp, :], in_=pav[0:1, 512:1024]
                )
            # 1/s on DVE; one pass over partitions 0:65 covers both s rows.
            # Off the ACT engine entirely so its table set never leaves exp.
            nc.vector.reciprocal_approx_fast(
                out=scolf[0:65, :, :], in_=scol[0:65, :, :]
            )
            nc.vector.tensor_copy(out=scolr[0:65, :, :], in_=scolf[0:65, :, :])
            for hp in range(4):
                rb = psum.tile([P, 1024], f32, tag="qk", bufs=2)
                nc.tensor.matmul(
                    rb[:, 0:512], bcm_sb[0:65, hp, :], scolr[0:65, hp, :],
                    start=True, stop=True,
                )
                nc.vector.tensor_mul(
                    out=yt_t[ib][:, hp, :],
                    in0=yt_t[ib][:, hp, :],
                    in1=rb[:, 0:512],
                )

            # ---- out-proj + store for block ib ----
            # blocks 1 and 2 are deferred behind stage 3's emission so
            # their matmuls fill the PE bubbles of the most ACT-bound stage
            def emit_outproj(ib):
                for t4 in range(4):
                    tcn = ib * 4 + t4
                    ob = work.tile([P, C], f32, tag="ob", bufs=2, name="ob")
                    pso = psum.tile([P, 1024], f32, tag="qk", bufs=2, name="pso")
                    for n2 in range(C // 512):
                        for dc in range(4):
                            nc.tensor.matmul(
                                pso[:, n2 * 512 : (n2 + 1) * 512],
                                yt_t[ib][:, dc, t4 * P : (t4 + 1) * P],
                                wp_sb[:, dc, n2 * 512 : (n2 + 1) * 512],
                                start=(dc == 0),
                                stop=(dc == 3),
                            )
                    nc.vector.tensor_copy(out=ob[:], in_=pso[:])
                    nc.sync.dma_start(out_r[:, tcn, :], ob[:])

            if ib in (1, 2):
                outproj_deferred.append(ib)
            else:
                emit_outproj(ib)
            if ib == nb - 1:
                for j in outproj_deferred:
                    emit_outproj(j)

    nc.finalize()
    return nc


_NC = None


def _get_nc():
    global _NC
    if _NC is None:
        _NC = build_nc()
    return _NC


def make_in_maps(x, Wk, Wq, Wv, Wp):
    x = np.asarray(x, dtype=np.float32)
    Wk = np.asarray(Wk, dtype=np.float32)
    Wq = np.asarray(Wq, dtype=np.float32)
    Wv = np.asarray(Wv, dtype=np.float32)
    Wp = np.asarray(Wp, dtype=np.float32)
    tri = np.triu(np.ones((P, P), np.float32)).astype(ml_dtypes.bfloat16)
    # bcm broadcasts 1/s rows (even at 64, odd at 0) to yt rows
    bcm = np.zeros((P, 4, P), np.float32)
    for hp in range(4):
        bcm[64, hp, 0:64] = 1.0
        bcm[0, hp, 64:128] = 1.0
    bcm = bcm.astype(ml_dtypes.bfloat16)
    in_maps = []
    for c in range(8):
        b, g = c // 2, c % 2
        sl = slice(g * D, (g + 1) * D)
        in_maps.append({
            "xt": np.ascontiguousarray(x[b].T).astype(ml_dtypes.bfloat16),
            "wq": np.ascontiguousarray(Wq[:, sl]).astype(ml_dtypes.bfloat16),
            "wk": np.ascontiguousarray(Wk[:, sl]).astype(ml_dtypes.bfloat16),
            "wv": np.ascontiguousarray(Wv[:, sl]).astype(ml_dtypes.bfloat16),
            "wp": np.ascontiguousarray(Wp[sl, :]).astype(ml_dtypes.bfloat16),
            "tri": tri,
            "bcm": bcm,
        })
    return in_maps


def _run(x, Wk, Wq, Wv, Wp, trace=False):
    nc = _get_nc()
    in_maps = make_in_maps(x, Wk, Wq, Wv, Wp)
    res = run_bass_kernel_spmd(nc, in_maps, core_ids=list(range(8)), trace=trace)
    parts = [res.results[c]["out"] for c in range(8)]
    out = np.stack(
        [parts[2 * b] + parts[2 * b + 1] for b in range(B)], axis=0
    ).astype(np.float32)
    return out, res


def kernel(x, Wk, Wq, Wv, Wp):
    out, _ = _run(x, Wk, Wq, Wv, Wp, trace=False)
    return out
